# revision 1
# baseline (speedup 1.0000x reference)
"""Trainium2 Bass kernel for nn_CompactBilinearPoolingTSP.

The count-sketch + FFT circular-convolution pipeline collapses, via Parseval,
into dense half-spectrum DFT matmuls: F1[r,k] = sum_c X[r,c] E1[c,k] with
E1[c,k] = s1[c] exp(-2i pi k h1[c] / D) a host-precomputed constant,
Phi = F1 * F2, and ip[r] = (1/D) sum_k gamma[k] Re(Phi conj(F1y F2y)).
The sensor branch is rank-1 in s, so the y-side spectra reduce to three
per-b vectors (t rows and a ones row are appended to X so one set of matmuls
produces every needed spectrum); a second small matmul contracts Phi against
them over k.  Tail (signed sqrt, L2 normalize over s, output projection) runs
on vector/scalar engines.  Sharding: pure data parallel, batch 32 -> 4 per
core across 8 NeuronCores.  All data-dependent compute runs on device; host
precomputes only hash-derived constants (E, gamma, V3) and input layout.
"""

import numpy as np

try:
    import concourse.bass  # noqa: F401
except ImportError:  # pragma: no cover
    import sys
    for _p in ("/opt/trn_rl_repo", "/root/.axon_site/_ro/trn_rl_repo"):
        if _p not in sys.path:
            sys.path.append(_p)

_PROGRAM = None

B, S, C, D, SN = 32, 145, 768, 8192, 64
NCORES = 8
BC = B // NCORES          # batches per core = 4
NRX = BC * S              # x rows per core = 580
NR = NRX + BC + 1         # + t rows + ones row = 585
KF = D // 2 + 1           # 4097 distinct freqs
NFT = 33                  # freq tiles of 128 -> 4224 padded
KP = NFT * 128
KT = C // 128             # 6 contraction tiles
CH = [(0, 293), (293, 292)]  # row chunks for matmul N


def _host_constants(h1, h2, s1, s2):
    """E matrices, gamma, V3 — all derived from hash/sign vectors only."""
    h1 = h1.astype(np.int64); h2 = h2.astype(np.int64)
    s1f = s1.astype(np.float64); s2f = s2.astype(np.float64)
    k = np.arange(KP)
    ang1 = (-2.0 * np.pi / D) * (h1[:, None] * k[None, :])
    ang2 = (-2.0 * np.pi / D) * (h2[:, None] * k[None, :])
    E1 = s1f[:, None] * np.exp(1j * ang1)
    E2 = s2f[:, None] * np.exp(1j * ang2)
    E1[:, KF:] = 0.0
    E2[:, KF:] = 0.0
    # planes: 0=E1r 1=E1i 2=E2r 3=E2i ; layout [NFT, 128k, KT, plane, 128f]
    E = np.stack([E1.real, E1.imag, E2.real, E2.imag], axis=0)  # [4, C, KP]
    E = E.reshape(4, KT, 128, NFT, 128)                          # [p, kt, k, ft, f]
    E = E.transpose(3, 2, 1, 0, 4)                               # [ft, k, kt, p, f]
    E = np.ascontiguousarray(E, dtype=np.float16)

    gamma = np.full(KP, 2.0)
    gamma[0] = 1.0
    gamma[KF - 1] = 1.0
    gamma[KF:] = 0.0
    gamma_sb = gamma.reshape(NFT, 128).T.astype(np.float32)      # [128, NFT]

    # V3 = gamma * (W3R, W3I), W3 = Q1*Q2, Q = ones @ E  (exact, host)
    Q1 = np.ones(C) @ E1
    Q2 = np.ones(C) @ E2
    W3 = Q1 * Q2
    v3 = np.stack([(gamma * W3.real), (gamma * W3.imag)], axis=-1)  # [KP, 2]
    v3_sb = v3.reshape(NFT, 128, 2).transpose(1, 0, 2)              # [128, NFT, 2]
    v3_sb = np.ascontiguousarray(v3_sb, dtype=np.float16)
    return E, gamma_sb, v3_sb


def _host_inputs_for_core(core, inputs, E, gamma_sb, v3_sb):
    """Per-core in_map (numpy) keyed by dram tensor names."""
    img = np.asarray(inputs["image_embeds"], np.float32)
    sensor = np.asarray(inputs["sensor"], np.float32)
    b0 = core * BC
    ximg = np.ascontiguousarray(img[b0:b0 + BC].reshape(NRX, C))
    sensT = np.ascontiguousarray(sensor[b0:b0 + BC, 0, :].T)     # [SN, BC]

    w2 = np.asarray(inputs["W_s2"], np.float32)[:, 0]            # [S]
    beta = np.asarray(inputs["b_s2"], np.float32)                # [S]
    wv = np.stack([w2 * w2, w2 * beta, beta * beta], 0) / D      # [3, S]
    wvec4 = np.ascontiguousarray(np.broadcast_to(wv[:, None, :], (3, BC, S)),
                                 np.float32)
    wout4 = np.ascontiguousarray(
        np.broadcast_to(np.asarray(inputs["W_out"], np.float32)[0][None, None, :],
                        (1, BC, S)))
    tokv = np.asarray(inputs["tok_emb"], np.float32)[1].reshape(KT, 128).T
    bsen = np.asarray(inputs["b_sensor"], np.float32).reshape(KT, 128).T
    wsensT = np.ascontiguousarray(np.asarray(inputs["W_sensor"], np.float32).T)

    return {
        "ximg": ximg,
        "sensT": sensT.astype(np.float16),
        "wsensT": wsensT.astype(np.float16),
        "bsen": np.ascontiguousarray(bsen),
        "tokv": np.ascontiguousarray(tokv),
        "Econst": E,
        "gammac": gamma_sb,
        "v3c": v3_sb,
        "wvec4": wvec4,
        "wout4": wout4,
        "bout": np.asarray(inputs["b_out"], np.float32).reshape(1, 1),
        "ident": np.eye(128, dtype=np.float16),
    }


def _build_program():
    import concourse.tile as tile
    from concourse import bacc, mybir

    f16 = mybir.dt.float16
    f32 = mybir.dt.float32
    OP = mybir.AluOpType
    AF = mybir.ActivationFunctionType

    nc = bacc.Bacc("TRN2", target_bir_lowering=False, debug=False,
                   num_devices=NCORES)

    ximg = nc.dram_tensor("ximg", [NRX, C], f32, kind="ExternalInput")
    sensT = nc.dram_tensor("sensT", [SN, BC], f16, kind="ExternalInput")
    wsensT = nc.dram_tensor("wsensT", [SN, C], f16, kind="ExternalInput")
    bsen = nc.dram_tensor("bsen", [128, KT], f32, kind="ExternalInput")
    tokv = nc.dram_tensor("tokv", [128, KT], f32, kind="ExternalInput")
    Ec = nc.dram_tensor("Econst", [NFT, 128, KT, 4, 128], f16,
                        kind="ExternalInput")
    gammac = nc.dram_tensor("gammac", [128, NFT], f32, kind="ExternalInput")
    v3c = nc.dram_tensor("v3c", [128, NFT, 2], f16, kind="ExternalInput")
    wvec4 = nc.dram_tensor("wvec4", [3, BC, S], f32, kind="ExternalInput")
    wout4 = nc.dram_tensor("wout4", [1, BC, S], f32, kind="ExternalInput")
    bout = nc.dram_tensor("bout", [1, 1], f32, kind="ExternalInput")
    ident = nc.dram_tensor("ident", [128, 128], f16, kind="ExternalInput")
    out_d = nc.dram_tensor("out", [1, BC], f32, kind="ExternalOutput")

    with tile.TileContext(nc) as tc:
        with (
            tc.tile_pool(name="const", bufs=1) as cp,
            tc.tile_pool(name="xload", bufs=2) as xp,
            tc.tile_pool(name="estream", bufs=2) as ep,
            tc.tile_pool(name="fplane", bufs=2) as fp,
            tc.tile_pool(name="vtmp", bufs=2) as vp,
            tc.tile_pool(name="phip", bufs=1) as pp,
        ):
            # ---- persistent tiles ----
            xt = cp.tile([128, KT, NR], f16)          # rows^T (c on partitions)
            phiR = pp.tile([128, NFT, NR], f16)
            phiI = pp.tile([128, NFT, NR], f16)
            fy = cp.tile([128, NFT, 4, 5], f16)       # spectra of t rows + ones
            vt = cp.tile([128, NFT, 2, BC, 3], f16)   # lhsT for pass 2
            gam = cp.tile([128, NFT], f32)
            v3s = cp.tile([128, NFT, 2], f16)
            idn = cp.tile([128, 128], f16)
            tok = cp.tile([128, KT], f32)
            bse = cp.tile([128, KT], f32)
            wv4 = cp.tile([3, BC, S], f32)
            wo4 = cp.tile([1, BC, S], f32)
            bo = cp.tile([1, 1], f32)
            sy = nc.sync
            sy.dma_start(idn[:], ident.ap())
            sy.dma_start(gam[:], gammac.ap())
            sy.dma_start(v3s[:], v3c.ap())
            sy.dma_start(tok[:], tokv.ap())
            sy.dma_start(bse[:], bsen.ap())
            sy.dma_start(wv4[:], wvec4.ap())
            sy.dma_start(wo4[:], wout4.ap())
            sy.dma_start(bo[:], bout.ap())

            with tc.tile_pool(name="eps", bufs=2, space="PSUM") as eps:
                # ---- build xt: transpose image rows (fp16), add tok emb ----
                n_rt = (NRX + 127) // 128
                for rt in range(n_rt):
                    r0 = rt * 128
                    nr = min(128, NRX - r0)
                    xsb = xp.tile([128, C], f32, tag="xsb")
                    nc.scalar.dma_start(xsb[:nr, :], ximg.ap()[r0:r0 + nr, :])
                    xh = xp.tile([128, C], f16, tag="xh")
                    nc.vector.tensor_copy(xh[:nr, :], xsb[:nr, :])
                    for kt in range(KT):
                        pst = eps.tile([128, 128], f16, tag="pst")
                        nc.tensor.transpose(
                            pst[:, :nr], xh[:nr, kt * 128:(kt + 1) * 128],
                            idn[:nr, :nr])
                        nc.vector.tensor_tensor(
                            xt[:, kt, r0:r0 + nr], pst[:, :nr],
                            tok[:, kt:kt + 1].to_broadcast((128, nr)), OP.add)
                # ---- sensor branch -> t rows (cols NRX..NRX+BC) ----
                ssb = xp.tile([SN, BC], f16, tag="ssb")
                wsb = xp.tile([SN, C], f16, tag="wsb")
                sy.dma_start(ssb[:], sensT.ap())
                sy.dma_start(wsb[:], wsensT.ap())
                for kt in range(KT):
                    pss = eps.tile([128, BC], f32, tag="pss")
                    nc.tensor.matmul(pss[:], wsb[:, kt * 128:(kt + 1) * 128],
                                     ssb[:], start=True, stop=True)
                    nc.vector.tensor_tensor(
                        xt[:, kt, NRX:NRX + BC], pss[:],
                        bse[:, kt:kt + 1].to_broadcast((128, BC)), OP.add)
                nc.gpsimd.memset(xt[:, :, NR - 1:NR], 1.0)

            # ---- main loop over frequency tiles ----
            VGROUPS = {7: (0, 8), 15: (8, 16), 23: (16, 24), 32: (24, NFT)}

            def build_v_group(g0, g1):
                ng = g1 - g0
                sl = slice(g0, g1)
                P1r = fy[:, sl, 0, 0:BC]; P1i = fy[:, sl, 1, 0:BC]
                P2r = fy[:, sl, 2, 0:BC]; P2i = fy[:, sl, 3, 0:BC]
                shp = (128, ng, BC)
                Q1r = fy[:, sl, 0, 4:5].to_broadcast(shp)
                Q1i = fy[:, sl, 1, 4:5].to_broadcast(shp)
                Q2r = fy[:, sl, 2, 4:5].to_broadcast(shp)
                Q2i = fy[:, sl, 3, 4:5].to_broadcast(shp)
                gb = gam[:, sl, None].to_broadcast(shp)
                va = vp.tile([128, 9, BC], f32, tag="va", name="va")[:, :ng, :]
                vb = vp.tile([128, 9, BC], f32, tag="vb", name="vb")[:, :ng, :]
                vc = vp.tile([128, 9, BC], f32, tag="vc", name="vc")[:, :ng, :]
                TT = nc.vector.tensor_tensor
                TT(va[:], P1r, P2r, OP.mult)
                TT(vb[:], P1i, P2i, OP.mult)
                TT(vc[:], va[:], vb[:], OP.subtract)
                TT(vt[:, sl, 0, :, 0], vc[:], gb, OP.mult)
                TT(va[:], P1r, P2i, OP.mult)
                TT(vb[:], P1i, P2r, OP.mult)
                TT(vc[:], va[:], vb[:], OP.add)
                TT(vt[:, sl, 1, :, 0], vc[:], gb, OP.mult)
                TT(va[:], P1r, Q2r, OP.mult)
                TT(vb[:], P1i, Q2i, OP.mult)
                TT(va[:], va[:], vb[:], OP.subtract)
                TT(vb[:], P2r, Q1r, OP.mult)
                TT(vc[:], P2i, Q1i, OP.mult)
                TT(vb[:], vb[:], vc[:], OP.subtract)
                TT(va[:], va[:], vb[:], OP.add)
                TT(vt[:, sl, 0, :, 1], va[:], gb, OP.mult)
                TT(va[:], P1r, Q2i, OP.mult)
                TT(vb[:], P1i, Q2r, OP.mult)
                TT(va[:], va[:], vb[:], OP.add)
                TT(vb[:], P2r, Q1i, OP.mult)
                TT(vc[:], P2i, Q1r, OP.mult)
                TT(vb[:], vb[:], vc[:], OP.add)
                TT(va[:], va[:], vb[:], OP.add)
                TT(vt[:, sl, 1, :, 1], va[:], gb, OP.mult)
                nc.vector.tensor_copy(
                    vt[:, sl, :, :, 2],
                    v3s[:, sl, :, None].to_broadcast((128, ng, 2, BC)))

            with tc.tile_pool(name="mps", bufs=8, space="PSUM") as mps:
                for ft in range(NFT):
                    et = ep.tile([128, KT, 4, 128], f16, tag="et")
                    sy.dma_start(et[:], Ec.ap()[ft])
                    ftile = fp.tile([128, 4, NR], f16, tag="ftile")
                    for p in range(4):
                        for (c0, nn) in CH:
                            ps = mps.tile([128, 293], f32, tag="mm")
                            for kt in range(KT):
                                nc.tensor.matmul(
                                    ps[:, :nn], et[:, kt, p, :],
                                    xt[:, kt, c0:c0 + nn],
                                    start=(kt == 0), stop=(kt == KT - 1))
                            if p < 2:
                                nc.scalar.copy(ftile[:, p, c0:c0 + nn],
                                               ps[:, :nn])
                            else:
                                nc.vector.tensor_copy(ftile[:, p, c0:c0 + nn],
                                                      ps[:, :nn])
                    # persist spectra of the 5 appended rows
                    nc.scalar.copy(fy[:, ft, :, :], ftile[:, :, NRX:NR])
                    # Phi = F1 * F2 (complex)
                    t1 = vp.tile([128, NR], f16, tag="t1")
                    t2 = vp.tile([128, NR], f16, tag="t2")
                    t3 = vp.tile([128, NR], f16, tag="t3")
                    t4 = vp.tile([128, NR], f16, tag="t4")
                    nc.vector.tensor_tensor(t1[:], ftile[:, 0, :], ftile[:, 2, :], OP.mult)
                    nc.vector.tensor_tensor(t2[:], ftile[:, 1, :], ftile[:, 3, :], OP.mult)
                    nc.vector.tensor_tensor(phiR[:, ft, :], t1[:], t2[:], OP.subtract)
                    nc.vector.tensor_tensor(t3[:], ftile[:, 0, :], ftile[:, 3, :], OP.mult)
                    nc.vector.tensor_tensor(t4[:], ftile[:, 1, :], ftile[:, 2, :], OP.mult)
                    nc.vector.tensor_tensor(phiI[:, ft, :], t3[:], t4[:], OP.add)
                    if ft in VGROUPS:
                        build_v_group(*VGROUPS[ft])


            # ---- pass 2: T = sum_k V^T Phi  -> [3, S] per b ----
            tsb = cp.tile([3, BC, S], f32)
            ip = vp.tile([1, BC, S], f32, tag="ip")
            with tc.tile_pool(name="p2ps", bufs=1, space="PSUM") as p2:
                tps = [p2.tile([3, S], f32, tag=f"tps{b}", name=f"tps{b}")
                       for b in range(BC)]
                for ft in range(NFT):
                    for b in range(BC):
                        nc.tensor.matmul(
                            tps[b][:], vt[:, ft, 0, b, :],
                            phiR[:, ft, b * S:(b + 1) * S],
                            start=(ft == 0), stop=False)
                        nc.tensor.matmul(
                            tps[b][:], vt[:, ft, 1, b, :],
                            phiI[:, ft, b * S:(b + 1) * S],
                            start=False, stop=(ft == NFT - 1))
                for b in range(BC):
                    nc.scalar.copy(tsb[:, b, :], tps[b][:])
                # ip = sum_j wvec[j] * T[j]  (partition reduce via ones matmul)
                uu = vp.tile([3, BC, S], f32, tag="uu")
                nc.vector.tensor_tensor(uu[:], tsb[:], wv4[:], OP.mult)
                one3 = cp.tile([3, 1], f32)
                nc.gpsimd.memset(one3[:], 1.0)
                for h in range(2):
                    ipp = p2.tile([1, 2 * S], f32, tag=f"ipp{h}",
                                  name=f"ipp{h}")
                    nc.tensor.matmul(
                        ipp[:], one3[:],
                        uu[:].rearrange("j b s -> j (b s)")[
                            :, h * 2 * S:(h + 1) * 2 * S],
                        start=True, stop=True)
                    nc.scalar.copy(
                        ip[:].rearrange("a b s -> a (b s)")[
                            :, h * 2 * S:(h + 1) * 2 * S], ipp[:])

            # ---- tail ----
            sgn = vp.tile([1, BC, S], f32, tag="sgn")
            nc.vector.tensor_scalar(sgn[:], ip[:], 0.0, None, OP.is_ge)
            nc.vector.tensor_scalar(sgn[:], sgn[:], 2.0, -1.0, OP.mult, OP.add)
            av = vp.tile([1, BC, S], f32, tag="av")
            nc.vector.tensor_tensor(av[:], ip[:], sgn[:], OP.mult)
            z11 = cp.tile([1, 1], f32)
            nc.gpsimd.memset(z11[:], 0.0)
            e11 = cp.tile([1, 1], f32)
            nc.gpsimd.memset(e11[:], 1e-5)
            sq = vp.tile([1, BC, S], f32, tag="sq")
            nc.scalar.activation(sq[:], av[:], AF.Sqrt, bias=e11[:])
            bp = vp.tile([1, BC, S], f32, tag="bp")
            nc.vector.tensor_tensor(bp[:], sq[:], sgn[:], OP.mult)
            n2 = vp.tile([1, BC], f32, tag="n2")
            sq2 = vp.tile([1, BC, S], f32, tag="sq2")
            nc.vector.tensor_tensor(sq2[:], bp[:], bp[:], OP.mult)
            for b in range(BC):
                nc.vector.tensor_reduce(n2[:, b:b + 1], sq2[:, b, :],
                                        axis=mybir.AxisListType.X, op=OP.add)
            nc.vector.tensor_scalar(n2[:], n2[:], 1e-24, None, OP.max)
            inv2 = vp.tile([1, BC], f32, tag="inv2")
            nc.vector.reciprocal(inv2[:], n2[:])
            invn = vp.tile([1, BC], f32, tag="invn")
            nc.scalar.activation(invn[:], inv2[:], AF.Sqrt, bias=z11[:])
            mm2 = vp.tile([1, BC, S], f32, tag="mm2")
            nc.vector.tensor_tensor(mm2[:], bp[:], wo4[:], OP.mult)
            ds = vp.tile([1, BC], f32, tag="ds")
            for b in range(BC):
                nc.vector.tensor_reduce(ds[:, b:b + 1], mm2[:, b, :],
                                        axis=mybir.AxisListType.X, op=OP.add)
            res = vp.tile([1, BC], f32, tag="res")
            nc.vector.tensor_tensor(res[:], ds[:], invn[:], OP.mult)
            nc.vector.tensor_tensor(res[:], res[:],
                                    bo[:, 0:1].to_broadcast((1, BC)), OP.add)
            sy.dma_start(out_d.ap(), res[:])

    nc.compile()
    return nc


def kernel(**inputs) -> np.ndarray:
    global _PROGRAM
    if _PROGRAM is None:
        _PROGRAM = _build_program()
    nc = _PROGRAM

    E, gamma_sb, v3_sb = _host_constants(
        inputs["h1"], inputs["h2"], inputs["s1"], inputs["s2"])
    in_maps = [_host_inputs_for_core(c, inputs, E, gamma_sb, v3_sb)
               for c in range(NCORES)]

    from concourse.bass_utils import run_bass_kernel_spmd
    res = run_bass_kernel_spmd(nc, in_maps, list(range(NCORES)))
    out = np.concatenate([res.results[c]["out"][0] for c in range(NCORES)],
                         axis=0)
    return out.reshape(B, 1).astype(np.float32)



# revision 20
# speedup vs baseline: 1.0391x; 1.0391x over previous
"""Trainium2 Bass kernel for nn_CompactBilinearPoolingTSP.

The count-sketch + FFT circular-convolution pipeline collapses, via Parseval,
into dense half-spectrum DFT matmuls: F1[r,k] = sum_c X[r,c] E1[c,k] with
E1[c,k] = s1[c] exp(-2i pi k h1[c] / D) a host-precomputed constant,
Phi = F1 * F2, and ip[r] = (1/D) sum_k gamma[k] Re(Phi conj(F1y F2y)).
The sensor branch is rank-1 in s, so the y-side spectra reduce to three
per-b vectors (t rows and a ones row are appended to X so one set of matmuls
produces every needed spectrum); a second small matmul contracts Phi against
them over k.  Tail (signed sqrt, L2 normalize over s, output projection) runs
on vector/scalar engines.  Sharding: pure data parallel, batch 32 -> 4 per
core across 8 NeuronCores.

v2.1: host supplies the image pre-transposed in f16 and DMAs it straight into
the xt tile (one transfer); small constants are packed into three dram
tensors to cut DMA-issue serialization at startup; the Nyquist frequency 4096
runs as a rank-1 side path hoisted before the main loop (it opens the pass-2
PSUM accumulation chains); pass 2 is interleaved into the main loop per
8-tile group so phi tiles never persist; PSUM evacuation runs on the scalar
engine; the sqrt table is prewarmed; the tail is fused (|ip|+eps via abs_max,
single-instruction per-b reductions).
"""

import numpy as np

try:
    import concourse.bass  # noqa: F401
except ImportError:  # pragma: no cover
    import sys
    for _p in ("/opt/trn_rl_repo", "/root/.axon_site/_ro/trn_rl_repo"):
        if _p not in sys.path:
            sys.path.append(_p)

_PROGRAM = None

B, S, C, D, SN = 32, 145, 768, 8192, 64
NCORES = 8
BC = B // NCORES          # batches per core = 4
NRX = BC * S              # x rows per core = 580
NR = NRX + BC + 1         # + t rows + ones row = 585
NFT = 32                  # full freq tiles of 128 -> 4096; k=4096 separate
KP = NFT * 128
KT = C // 128             # 6 contraction tiles
CH = [(0, 293), (293, 292)]  # row chunks for matmul N
VG = {7: (0, 8), 15: (8, 16), 23: (16, 24), 31: (24, 32)}


def _host_constants(h1, h2, s1, s2):
    """E matrices + packed small constants — derived from hashes only."""
    h1 = h1.astype(np.int64); h2 = h2.astype(np.int64)
    s1f = s1.astype(np.float64); s2f = s2.astype(np.float64)
    k = np.arange(KP)
    ang1 = (-2.0 * np.pi / D) * (h1[:, None] * k[None, :])
    ang2 = (-2.0 * np.pi / D) * (h2[:, None] * k[None, :])
    E1 = s1f[:, None] * np.exp(1j * ang1)
    E2 = s2f[:, None] * np.exp(1j * ang2)
    # planes: 0=E1r 1=E1i 2=E2r 3=E2i ; layout [NFT, 128c, KT, plane, 128f]
    E = np.stack([E1.real, E1.imag, E2.real, E2.imag], axis=0)  # [4, C, KP]
    E = E.reshape(4, KT, 128, NFT, 128)                          # [p, kt, c, ft, f]
    E = E.transpose(3, 2, 1, 0, 4)                               # [ft, c, kt, p, f]
    E = np.ascontiguousarray(E, dtype=np.float16)

    gamma = np.full(KP, 2.0)
    gamma[0] = 1.0
    gamma_sb = gamma.reshape(NFT, 128).T.astype(np.float32)      # [128, NFT]

    # v3 = gamma * (W3R, W3I), W3 = Q1*Q2, Q = ones @ E  (exact, host)
    Q1 = np.ones(C) @ E1
    Q2 = np.ones(C) @ E2
    W3 = Q1 * Q2
    v3 = np.stack([(gamma * W3.real), (gamma * W3.imag)], axis=-1)  # [KP, 2]
    v3_sb = v3.reshape(NFT, 128, 2).transpose(1, 0, 2)              # [128, NFT, 2]

    # Nyquist (k = D/2): E real, gamma = 1
    e1n = s1f * np.where(h1 % 2 == 0, 1.0, -1.0)
    e2n = s2f * np.where(h2 % 2 == 0, 1.0, -1.0)
    eny = np.stack([e1n, e2n], axis=-1).reshape(KT, 128, 2).transpose(1, 0, 2)
    qn = np.array([e1n.sum(), e2n.sum()], np.float64)               # Q1n, Q2n

    # f16 pack [128, 2*NFT + 2*KT]: v3 | eny
    pk16 = np.concatenate([v3_sb.reshape(128, 2 * NFT),
                           eny.reshape(128, 2 * KT)], axis=1)
    pk16 = np.ascontiguousarray(pk16, np.float16)
    return E, gamma_sb, pk16, qn


def _host_inputs_for_core(core, inputs, consts):
    """Per-core in_map (numpy) keyed by dram tensor names."""
    E, gamma_sb, pk16, qn = consts
    img = np.asarray(inputs["image_embeds"], np.float32)
    sensor = np.asarray(inputs["sensor"], np.float32)
    b0 = core * BC
    # [128, KT, NRX]: ximg2[p, kt, r] = img_row_r[kt*128 + p]
    ximg2 = np.ascontiguousarray(
        img[b0:b0 + BC].reshape(NRX, C).T.reshape(KT, 128, NRX)
        .transpose(1, 0, 2).astype(np.float16))

    w2 = np.asarray(inputs["W_s2"], np.float32)[:, 0]            # [S]
    beta = np.asarray(inputs["b_s2"], np.float32)                # [S]
    wv = np.stack([w2 * w2, w2 * beta, beta * beta], 0) / D      # [3, S]
    wv4 = np.broadcast_to(wv[:, None, :], (3, BC, S)).reshape(3, NRX)
    wo4 = np.broadcast_to(np.asarray(inputs["W_out"], np.float32)[0][None, :],
                          (BC, S)).reshape(NRX)
    # pk4 [3, 2*NRX + 4]: rows 0-2 cols :NRX = wv4; row 0 cols NRX: = wo4 |
    # qn1 qn2 v3n bout (row 0 so every 1-partition operand sits at part 0)
    pk4 = np.zeros((3, 2 * NRX + 4), np.float32)
    pk4[0:3, :NRX] = wv4
    pk4[0, NRX:2 * NRX] = wo4
    pk4[0, 2 * NRX + 0] = qn[0]
    pk4[0, 2 * NRX + 1] = qn[1]
    pk4[0, 2 * NRX + 2] = np.float32(qn[0] * qn[1])
    pk4[0, 2 * NRX + 3] = np.asarray(inputs["b_out"], np.float32).ravel()[0]

    tokv = np.asarray(inputs["tok_emb"], np.float32)[1].reshape(KT, 128).T
    bsen = np.asarray(inputs["b_sensor"], np.float32).reshape(KT, 128).T
    pk32 = np.concatenate([gamma_sb, tokv, bsen], axis=1)        # [128, NFT+12]

    # sensor pack [SN, C + BC] f16: wsensT | sensT
    wsensT = np.asarray(inputs["W_sensor"], np.float32).T        # [SN, C]
    sensT = sensor[b0:b0 + BC, 0, :].T                           # [SN, BC]
    pksen = np.ascontiguousarray(
        np.concatenate([wsensT, sensT], axis=1).astype(np.float16))

    return {
        "ximg2": ximg2,
        "Econst": E,
        "pk32": np.ascontiguousarray(pk32, np.float32),
        "pk16": pk16,
        "pk4": np.ascontiguousarray(pk4),
        "pksen": pksen,
    }


def _build_program():
    import concourse.tile as tile
    from concourse import bacc, mybir

    f16 = mybir.dt.float16
    f32 = mybir.dt.float32
    OP = mybir.AluOpType
    AF = mybir.ActivationFunctionType

    nc = bacc.Bacc("TRN2", target_bir_lowering=False, debug=False,
                   num_devices=NCORES)

    ximg2 = nc.dram_tensor("ximg2", [128, KT, NRX], f16, kind="ExternalInput")
    Ec = nc.dram_tensor("Econst", [NFT, 128, KT, 4, 128], f16,
                        kind="ExternalInput")
    pk32d = nc.dram_tensor("pk32", [128, NFT + 2 * KT], f32,
                           kind="ExternalInput")
    pk16d = nc.dram_tensor("pk16", [128, 2 * NFT + 2 * KT], f16,
                           kind="ExternalInput")
    pk4d = nc.dram_tensor("pk4", [3, 2 * NRX + 4], f32, kind="ExternalInput")
    pksend = nc.dram_tensor("pksen", [SN, C + BC], f16, kind="ExternalInput")
    out_d = nc.dram_tensor("out", [1, BC], f32, kind="ExternalOutput")

    with tile.TileContext(nc) as tc:
        with (
            tc.tile_pool(name="const", bufs=1) as cp,
            tc.tile_pool(name="estream", bufs=3) as ep,
            tc.tile_pool(name="fplane", bufs=2) as fp,
            tc.tile_pool(name="phip", bufs=10) as php,
            tc.tile_pool(name="vtmp", bufs=2) as vp,
        ):
            # ---- persistent tiles ----
            xt = cp.tile([128, KT, NR], f16)          # rows^T (c on partitions)
            fy = cp.tile([128, NFT, 4, 5], f16)       # spectra of t rows + ones
            vt = cp.tile([128, NFT, 2, BC, 3], f16)   # lhsT for pass 2
            pk32 = cp.tile([128, NFT + 2 * KT], f32)
            pk16 = cp.tile([128, 2 * NFT + 2 * KT], f16)
            pk4 = cp.tile([3, 2 * NRX + 4], f32)
            pksen = cp.tile([SN, C + BC], f16)
            z11 = cp.tile([1, 1], f32)
            e11 = cp.tile([1, 1], f32)
            one3 = cp.tile([3, 1], f16)
            sy = nc.sync
            gam = pk32[:, 0:NFT]
            tok = pk32[:, NFT:NFT + KT]
            bse = pk32[:, NFT + KT:NFT + 2 * KT]
            v3s = pk16[:, 0:2 * NFT].rearrange("p (ft two) -> p ft two", two=2)
            eny = pk16[:, 2 * NFT:].rearrange("p (kt two) -> p kt two", two=2)
            wv4 = pk4[0:3, 0:NRX].rearrange("j (b s) -> j b s", b=BC)
            wo4 = pk4[0:1, NRX:2 * NRX].rearrange("j (b s) -> j b s", b=BC)
            qn3 = pk4[0:1, 2 * NRX:2 * NRX + 3]
            bo = pk4[0:1, 2 * NRX + 3:2 * NRX + 4]
            # image straight into xt (strided dst: 580 of 585 cols per kt)
            sy.dma_start(xt[:, :, 0:NRX], ximg2.ap())
            sy.dma_start(pk32[:], pk32d.ap())
            sy.dma_start(pk16[:], pk16d.ap())
            sy.dma_start(pk4[:], pk4d.ap())
            sy.dma_start(pksen[:], pksend.ap())
            nc.gpsimd.memset(z11[:], 0.0)
            nc.gpsimd.memset(e11[:], 1e-5)
            nc.gpsimd.memset(one3[:], 1.0)
            nc.gpsimd.memset(xt[:, :, NR - 1:NR], 1.0)
            # prewarm the Sqrt activation table so the tail doesn't stall
            warm = vp.tile([1, 1], f32, tag="warm")
            nc.scalar.activation(warm[:], z11[:], AF.Sqrt, bias=z11[:])

            # ---- xt finalize: add tok emb in place ----
            for kt in range(KT):
                nc.vector.tensor_tensor(
                    xt[:, kt, 0:NRX], xt[:, kt, 0:NRX],
                    tok[:, kt:kt + 1].to_broadcast((128, NRX)), OP.add)

            def build_v_group(g0, g1):
                ng = g1 - g0
                sl = slice(g0, g1)
                P1r = fy[:, sl, 0, 0:BC]; P1i = fy[:, sl, 1, 0:BC]
                P2r = fy[:, sl, 2, 0:BC]; P2i = fy[:, sl, 3, 0:BC]
                shp = (128, ng, BC)
                Q1r = fy[:, sl, 0, 4:5].to_broadcast(shp)
                Q1i = fy[:, sl, 1, 4:5].to_broadcast(shp)
                Q2r = fy[:, sl, 2, 4:5].to_broadcast(shp)
                Q2i = fy[:, sl, 3, 4:5].to_broadcast(shp)
                gb = gam[:, sl, None].to_broadcast(shp)
                va = vp.tile([128, 8, BC], f32, tag="va", name="va")[:, :ng, :]
                vb = vp.tile([128, 8, BC], f32, tag="vb", name="vb")[:, :ng, :]
                vc = vp.tile([128, 8, BC], f32, tag="vc", name="vc")[:, :ng, :]
                TT = nc.vector.tensor_tensor
                TT(va[:], P1r, P2r, OP.mult)
                TT(vb[:], P1i, P2i, OP.mult)
                TT(vc[:], va[:], vb[:], OP.subtract)
                TT(vt[:, sl, 0, :, 0], vc[:], gb, OP.mult)
                TT(va[:], P1r, P2i, OP.mult)
                TT(vb[:], P1i, P2r, OP.mult)
                TT(vc[:], va[:], vb[:], OP.add)
                TT(vt[:, sl, 1, :, 0], vc[:], gb, OP.mult)
                TT(va[:], P1r, Q2r, OP.mult)
                TT(vb[:], P1i, Q2i, OP.mult)
                TT(va[:], va[:], vb[:], OP.subtract)
                TT(vb[:], P2r, Q1r, OP.mult)
                TT(vc[:], P2i, Q1i, OP.mult)
                TT(vb[:], vb[:], vc[:], OP.subtract)
                TT(va[:], va[:], vb[:], OP.add)
                TT(vt[:, sl, 0, :, 1], va[:], gb, OP.mult)
                TT(va[:], P1r, Q2i, OP.mult)
                TT(vb[:], P1i, Q2r, OP.mult)
                TT(va[:], va[:], vb[:], OP.add)
                TT(vb[:], P2r, Q1i, OP.mult)
                TT(vc[:], P2i, Q1r, OP.mult)
                TT(vb[:], vb[:], vc[:], OP.add)
                TT(va[:], va[:], vb[:], OP.add)
                TT(vt[:, sl, 1, :, 1], va[:], gb, OP.mult)
                nc.vector.tensor_copy(
                    vt[:, sl, :, :, 2],
                    v3s[:, sl, :, None].to_broadcast((128, ng, 2, BC)))

            with (
                tc.tile_pool(name="mps", bufs=4, space="PSUM") as mps,
                tc.tile_pool(name="p2ps", bufs=1, space="PSUM") as p2,
            ):
                # sensor branch -> t rows (cols NRX..NRX+BC)
                for kt in range(KT):
                    pss = mps.tile([128, 293], f32, tag="mm",
                                   name="pss")[:, 0:BC]
                    nc.tensor.matmul(pss[:],
                                     pksen[:, kt * 128:(kt + 1) * 128],
                                     pksen[:, C:C + BC], start=True, stop=True)
                    nc.vector.tensor_tensor(
                        xt[:, kt, NRX:NRX + BC], pss[:],
                        bse[:, kt:kt + 1].to_broadcast((128, BC)), OP.add)

                tps = [p2.tile([3, S], f32, tag=f"tps{b}", name=f"tps{b}")
                       for b in range(BC)]

                # ---- Nyquist k=4096 (real spectra, gamma=1): opens the
                # pass-2 accumulation chains with a rank-1 update ----
                fn0 = fp.tile([1, NR], f16, tag="fn0")
                fn1 = fp.tile([1, NR], f16, tag="fn1")
                for pl, fn in ((0, fn0), (1, fn1)):
                    for (c0, nn) in CH:
                        psn = mps.tile([128, 293], f32, tag="mm",
                                       name="psn")[0:1, :]
                        for kt in range(KT):
                            nc.tensor.matmul(
                                psn[:, :nn], eny[:, kt, pl:pl + 1],
                                xt[:, kt, c0:c0 + nn],
                                start=(kt == 0), stop=(kt == KT - 1))
                        nc.scalar.copy(fn[0:1, c0:c0 + nn], psn[:, :nn])
                phin = vp.tile([1, NR], f16, tag="phin")
                nc.vector.tensor_tensor(phin[:], fn0[:], fn1[:], OP.mult)
                # vtn[b, j]: j0 = T1n*T2n, j1 = T1n*Q2n + T2n*Q1n, j2 = Q1n*Q2n
                vtn = vp.tile([1, BC, 3], f16, tag="vtn")
                ta = vp.tile([1, BC], f32, tag="ta")
                tb = vp.tile([1, BC], f32, tag="tb")
                nc.vector.tensor_tensor(ta[:], fn0[0:1, NRX:NRX + BC],
                                        fn1[0:1, NRX:NRX + BC], OP.mult)
                nc.vector.tensor_copy(vtn[:, :, 0], ta[:])
                nc.vector.tensor_tensor(
                    ta[:], fn0[0:1, NRX:NRX + BC],
                    qn3[:, 1:2].to_broadcast((1, BC)), OP.mult)
                nc.vector.tensor_tensor(
                    tb[:], fn1[0:1, NRX:NRX + BC],
                    qn3[:, 0:1].to_broadcast((1, BC)), OP.mult)
                nc.vector.tensor_tensor(ta[:], ta[:], tb[:], OP.add)
                nc.vector.tensor_copy(vtn[:, :, 1], ta[:])
                nc.vector.tensor_copy(
                    vtn[:, :, 2], qn3[:, 2:3].to_broadcast((1, BC)))
                for b in range(BC):
                    nc.tensor.matmul(
                        tps[b][:], vtn[0:1, b, :],
                        phin[0:1, b * S:(b + 1) * S],
                        start=True, stop=False)

                # ---- main loop over frequency tiles; pass 2 per group ----
                phis = {}
                for ft in range(NFT):
                    et = ep.tile([128, KT, 4, 128], f16, tag="et")
                    sy.dma_start(et[:], Ec.ap()[ft])
                    ftile = fp.tile([128, 4, NR], f16, tag="ftile")
                    for p in range(4):
                        for (c0, nn) in CH:
                            ps = mps.tile([128, 293], f32, tag="mm")
                            for kt in range(KT):
                                nc.tensor.matmul(
                                    ps[:, :nn], et[:, kt, p, :],
                                    xt[:, kt, c0:c0 + nn],
                                    start=(kt == 0), stop=(kt == KT - 1))
                            nc.scalar.copy(ftile[:, p, c0:c0 + nn],
                                           ps[:, :nn])
                    # persist spectra of the 5 appended rows
                    nc.scalar.copy(fy[:, ft, :, :], ftile[:, :, NRX:NR])
                    # Phi = F1 * F2 (complex)
                    t1 = vp.tile([128, NR], f16, tag="t1")
                    t2 = vp.tile([128, NR], f16, tag="t2")
                    phR = php.tile([128, NR], f16, tag="phR")
                    phI = php.tile([128, NR], f16, tag="phI")
                    TT = nc.vector.tensor_tensor
                    TT(t1[:], ftile[:, 0, :], ftile[:, 2, :], OP.mult)
                    TT(t2[:], ftile[:, 1, :], ftile[:, 3, :], OP.mult)
                    TT(phR[:], t1[:], t2[:], OP.subtract)
                    TT(t1[:], ftile[:, 0, :], ftile[:, 3, :], OP.mult)
                    TT(t2[:], ftile[:, 1, :], ftile[:, 2, :], OP.mult)
                    TT(phI[:], t1[:], t2[:], OP.add)
                    phis[ft] = (phR, phI)
                    if ft in VG:
                        g0, g1 = VG[ft]
                        build_v_group(g0, g1)
                        for fti in range(g0, g1):
                            pR, pI = phis.pop(fti)
                            fin = fti == NFT - 1
                            for b in range(BC):
                                nc.tensor.matmul(
                                    tps[b][:], vt[:, fti, 0, b, :],
                                    pR[:, b * S:(b + 1) * S],
                                    start=False, stop=False)
                                nc.tensor.matmul(
                                    tps[b][:], vt[:, fti, 1, b, :],
                                    pI[:, b * S:(b + 1) * S],
                                    start=False, stop=fin)

                # ---- epilogue: combine j terms via ones matmul (f16) ----
                tsb = cp.tile([3, BC, S], f32)
                for b in range(BC):
                    nc.scalar.copy(tsb[:, b, :], tps[b][:])
                uu = vp.tile([3, BC, S], f16, tag="uu")
                nc.vector.tensor_tensor(uu[:], tsb[:], wv4[:], OP.mult)
                ip = vp.tile([1, BC, S], f32, tag="ip")
                for h in range(2):
                    ipp = mps.tile([128, 293], f32, tag="mm",
                                   name="ipp")[0:1, 0:2 * S]
                    nc.tensor.matmul(
                        ipp[:], one3[:],
                        uu[:].rearrange("j b s -> j (b s)")[
                            :, h * 2 * S:(h + 1) * 2 * S],
                        start=True, stop=True)
                    nc.scalar.copy(
                        ip[:].rearrange("a b s -> a (b s)")[
                            :, h * 2 * S:(h + 1) * 2 * S], ipp[:])

            # ---- tail: signed sqrt, L2 normalize over s, project ----
            sgn = vp.tile([1, BC, S], f32, tag="sgn")
            nc.vector.tensor_scalar(sgn[:], ip[:], 0.0, None, OP.is_ge)
            nc.vector.tensor_scalar(sgn[:], sgn[:], 2.0, -1.0, OP.mult, OP.add)
            av = vp.tile([1, BC, S], f32, tag="av")          # |ip|
            nc.vector.tensor_tensor(av[:], ip[:], sgn[:], OP.mult)
            sq = vp.tile([1, BC, S], f32, tag="sq")          # sqrt(|ip|+1e-5)
            nc.scalar.activation(sq[:], av[:], AF.Sqrt, bias=e11[:])
            gg = vp.tile([1, BC, S], f32, tag="gg")
            nc.vector.tensor_tensor(gg[:], sgn[:], wo4[:], OP.mult)
            mm2 = vp.tile([1, BC, S], f32, tag="mm2")
            nc.vector.tensor_tensor(mm2[:], sq[:], gg[:], OP.mult)
            n2 = vp.tile([1, BC], f32, tag="n2")
            ds = vp.tile([1, BC], f32, tag="ds")
            nc.vector.tensor_reduce(n2[:], av[:],
                                    axis=mybir.AxisListType.X, op=OP.add)
            # ||bp||^2 = sum(|ip|) + S*1e-5
            nc.vector.tensor_scalar(n2[:], n2[:], S * 1e-5, None, OP.add)
            nc.vector.tensor_reduce(ds[:], mm2[:],
                                    axis=mybir.AxisListType.X, op=OP.add)
            inv2 = vp.tile([1, BC], f32, tag="inv2")
            nc.vector.reciprocal(inv2[:], n2[:])
            invn = vp.tile([1, BC], f32, tag="invn")
            nc.scalar.activation(invn[:], inv2[:], AF.Sqrt, bias=z11[:])
            res = vp.tile([1, BC], f32, tag="res")
            nc.vector.tensor_tensor(res[:], ds[:], invn[:], OP.mult)
            nc.vector.tensor_tensor(res[:], res[:],
                                    bo[:, 0:1].to_broadcast((1, BC)), OP.add)
            sy.dma_start(out_d.ap(), res[:])

    nc.compile()
    return nc


def kernel(**inputs) -> np.ndarray:
    global _PROGRAM
    if _PROGRAM is None:
        _PROGRAM = _build_program()
    nc = _PROGRAM

    consts = _host_constants(
        inputs["h1"], inputs["h2"], inputs["s1"], inputs["s2"])
    in_maps = [_host_inputs_for_core(c, inputs, consts)
               for c in range(NCORES)]

    from concourse.bass_utils import run_bass_kernel_spmd
    res = run_bass_kernel_spmd(nc, in_maps, list(range(NCORES)))
    out = np.concatenate([res.results[c]["out"][0] for c in range(NCORES)],
                         axis=0)
    return out.reshape(B, 1).astype(np.float32)


# revision 23
# speedup vs baseline: 1.0568x; 1.0170x over previous
"""Trainium2 Bass kernel for nn_CompactBilinearPoolingTSP.

The count-sketch + FFT circular-convolution pipeline collapses, via Parseval,
into dense half-spectrum DFT matmuls: F1[r,k] = sum_c X[r,c] E1[c,k] with
E1[c,k] = s1[c] exp(-2i pi k h1[c] / D) a host-precomputed constant,
Phi = F1 * F2, and ip[r] = (1/D) sum_k gamma[k] Re(Phi conj(F1y F2y)).
The sensor branch is rank-1 in s, so the y-side spectra reduce to three
per-b vectors (t rows and a ones row are appended to X so one set of matmuls
produces every needed spectrum); a second small matmul contracts Phi against
them over k.  Tail (signed sqrt, L2 normalize over s, output projection) runs
on vector/scalar engines.  Sharding: pure data parallel, batch 32 -> 4 per
core across 8 NeuronCores.

v2.1: host supplies the image pre-transposed in f16 and DMAs it straight into
the xt tile (one transfer); small constants are packed into three dram
tensors to cut DMA-issue serialization at startup; the Nyquist frequency 4096
runs as a rank-1 side path hoisted before the main loop (it opens the pass-2
PSUM accumulation chains); pass 2 is interleaved into the main loop per
8-tile group so phi tiles never persist; PSUM evacuation runs on the scalar
engine; the sqrt table is prewarmed; the tail is fused (|ip|+eps via abs_max,
single-instruction per-b reductions).
"""

import numpy as np

try:
    import concourse.bass  # noqa: F401
except ImportError:  # pragma: no cover
    import sys
    for _p in ("/opt/trn_rl_repo", "/root/.axon_site/_ro/trn_rl_repo"):
        if _p not in sys.path:
            sys.path.append(_p)

_PROGRAM = None

B, S, C, D, SN = 32, 145, 768, 8192, 64
NCORES = 8
BC = B // NCORES          # batches per core = 4
NRX = BC * S              # x rows per core = 580
NR = NRX + BC + 1         # + t rows + ones row = 585
NFT = 32                  # full freq tiles of 128 -> 4096; k=4096 separate
KP = NFT * 128
KT = C // 128             # 6 contraction tiles
CH = [(0, 293), (293, 292)]  # row chunks for matmul N
VG = {7: (0, 8), 15: (8, 16), 23: (16, 24), 29: (24, 30), 31: (30, 32)}


def _host_constants(h1, h2, s1, s2):
    """E matrices + packed small constants — derived from hashes only."""
    h1 = h1.astype(np.int64); h2 = h2.astype(np.int64)
    s1f = s1.astype(np.float64); s2f = s2.astype(np.float64)
    k = np.arange(KP)
    ang1 = (-2.0 * np.pi / D) * (h1[:, None] * k[None, :])
    ang2 = (-2.0 * np.pi / D) * (h2[:, None] * k[None, :])
    E1 = s1f[:, None] * np.exp(1j * ang1)
    E2 = s2f[:, None] * np.exp(1j * ang2)
    # planes: 0=E1r 1=E1i 2=E2r 3=E2i ; layout [NFT, 128c, KT, plane, 128f]
    E = np.stack([E1.real, E1.imag, E2.real, E2.imag], axis=0)  # [4, C, KP]
    E = E.reshape(4, KT, 128, NFT, 128)                          # [p, kt, c, ft, f]
    E = E.transpose(3, 2, 1, 0, 4)                               # [ft, c, kt, p, f]
    E = np.ascontiguousarray(E, dtype=np.float16)

    gamma = np.full(KP, 2.0)
    gamma[0] = 1.0
    gamma_sb = gamma.reshape(NFT, 128).T.astype(np.float32)      # [128, NFT]

    # v3 = gamma * (W3R, W3I), W3 = Q1*Q2, Q = ones @ E  (exact, host)
    Q1 = np.ones(C) @ E1
    Q2 = np.ones(C) @ E2
    W3 = Q1 * Q2
    v3 = np.stack([(gamma * W3.real), (gamma * W3.imag)], axis=-1)  # [KP, 2]
    v3_sb = v3.reshape(NFT, 128, 2).transpose(1, 0, 2)              # [128, NFT, 2]

    # Nyquist (k = D/2): E real, gamma = 1
    e1n = s1f * np.where(h1 % 2 == 0, 1.0, -1.0)
    e2n = s2f * np.where(h2 % 2 == 0, 1.0, -1.0)
    eny = np.stack([e1n, e2n], axis=-1).reshape(KT, 128, 2).transpose(1, 0, 2)
    qn = np.array([e1n.sum(), e2n.sum()], np.float64)               # Q1n, Q2n

    # f16 pack [128, 2*NFT + 2*KT]: v3 | eny
    pk16 = np.concatenate([v3_sb.reshape(128, 2 * NFT),
                           eny.reshape(128, 2 * KT)], axis=1)
    pk16 = np.ascontiguousarray(pk16, np.float16)
    return E, gamma_sb, pk16, qn


def _host_inputs_for_core(core, inputs, consts):
    """Per-core in_map (numpy) keyed by dram tensor names."""
    E, gamma_sb, pk16, qn = consts
    img = np.asarray(inputs["image_embeds"], np.float32)
    sensor = np.asarray(inputs["sensor"], np.float32)
    b0 = core * BC
    # [128, KT, NRX]: ximg2[p, kt, r] = img_row_r[kt*128 + p]
    ximg2 = np.ascontiguousarray(
        img[b0:b0 + BC].reshape(NRX, C).T.reshape(KT, 128, NRX)
        .transpose(1, 0, 2).astype(np.float16))

    w2 = np.asarray(inputs["W_s2"], np.float32)[:, 0]            # [S]
    beta = np.asarray(inputs["b_s2"], np.float32)                # [S]
    wv = np.stack([w2 * w2, w2 * beta, beta * beta], 0) / D      # [3, S]
    wo4 = np.broadcast_to(np.asarray(inputs["W_out"], np.float32)[0][None, :],
                          (BC, S)).reshape(NRX)
    # pk4 [12, 2*NRX + 4]: rows (3b+j) hold wv[j] in batch b's column block of
    # each 290-wide pass-2 chunk (zeros elsewhere mask the off-block garbage
    # of the batched 12-wide pass-2 matmul); row 0 cols NRX: = wo4 | qn1 qn2
    # v3n bout (all 1-partition operands at partition 0)
    pk4 = np.zeros((12, 2 * NRX + 4), np.float32)
    for b in range(BC):
        ch, off = divmod(b * S, 2 * S)          # chunk index, offset in chunk
        for j in range(3):
            pk4[3 * b + j, ch * 2 * S + off:ch * 2 * S + off + S] = wv[j]
    pk4[0, NRX:2 * NRX] = wo4
    pk4[0, 2 * NRX + 0] = qn[0]
    pk4[0, 2 * NRX + 1] = qn[1]
    pk4[0, 2 * NRX + 2] = np.float32(qn[0] * qn[1])
    pk4[0, 2 * NRX + 3] = np.asarray(inputs["b_out"], np.float32).ravel()[0]

    tokv = np.asarray(inputs["tok_emb"], np.float32)[1].reshape(KT, 128).T
    bsen = np.asarray(inputs["b_sensor"], np.float32).reshape(KT, 128).T
    pk32 = np.concatenate([gamma_sb, tokv, bsen], axis=1)        # [128, NFT+12]

    # sensor pack [SN, C + BC] f16: wsensT | sensT
    wsensT = np.asarray(inputs["W_sensor"], np.float32).T        # [SN, C]
    sensT = sensor[b0:b0 + BC, 0, :].T                           # [SN, BC]
    pksen = np.ascontiguousarray(
        np.concatenate([wsensT, sensT], axis=1).astype(np.float16))

    return {
        "ximg2": ximg2,
        "Econst": E,
        "pk32": np.ascontiguousarray(pk32, np.float32),
        "pk16": pk16,
        "pk4": np.ascontiguousarray(pk4),
        "pksen": pksen,
    }


def _build_program():
    import concourse.tile as tile
    from concourse import bacc, mybir

    f16 = mybir.dt.float16
    f32 = mybir.dt.float32
    OP = mybir.AluOpType
    AF = mybir.ActivationFunctionType

    nc = bacc.Bacc("TRN2", target_bir_lowering=False, debug=False,
                   num_devices=NCORES)

    ximg2 = nc.dram_tensor("ximg2", [128, KT, NRX], f16, kind="ExternalInput")
    Ec = nc.dram_tensor("Econst", [NFT, 128, KT, 4, 128], f16,
                        kind="ExternalInput")
    pk32d = nc.dram_tensor("pk32", [128, NFT + 2 * KT], f32,
                           kind="ExternalInput")
    pk16d = nc.dram_tensor("pk16", [128, 2 * NFT + 2 * KT], f16,
                           kind="ExternalInput")
    pk4d = nc.dram_tensor("pk4", [12, 2 * NRX + 4], f32, kind="ExternalInput")
    pksend = nc.dram_tensor("pksen", [SN, C + BC], f16, kind="ExternalInput")
    out_d = nc.dram_tensor("out", [1, BC], f32, kind="ExternalOutput")

    with tile.TileContext(nc) as tc:
        with (
            tc.tile_pool(name="const", bufs=1) as cp,
            tc.tile_pool(name="estream", bufs=3) as ep,
            tc.tile_pool(name="fplane", bufs=2) as fp,
            tc.tile_pool(name="phip", bufs=10) as php,
            tc.tile_pool(name="vtmp", bufs=2) as vp,
        ):
            # ---- persistent tiles ----
            xt = cp.tile([128, KT, NR], f16)          # rows^T (c on partitions)
            fy = cp.tile([128, NFT, 4, 5], f16)       # spectra of t rows + ones
            vt = cp.tile([128, NFT, 2, BC, 3], f16)   # lhsT for pass 2
            pk32 = cp.tile([128, NFT + 2 * KT], f32)
            pk16 = cp.tile([128, 2 * NFT + 2 * KT], f16)
            pk4 = cp.tile([12, 2 * NRX + 4], f32)
            pksen = cp.tile([SN, C + BC], f16)
            z11 = cp.tile([1, 1], f32)
            e11 = cp.tile([1, 1], f32)
            one3 = cp.tile([12, 1], f16)
            sy = nc.sync
            gam = pk32[:, 0:NFT]
            tok = pk32[:, NFT:NFT + KT]
            bse = pk32[:, NFT + KT:NFT + 2 * KT]
            v3s = pk16[:, 0:2 * NFT].rearrange("p (ft two) -> p ft two", two=2)
            eny = pk16[:, 2 * NFT:].rearrange("p (kt two) -> p kt two", two=2)
            wv12 = pk4[0:12, 0:NRX].rearrange("p (h c) -> p h c", h=2)
            wo4 = pk4[0:1, NRX:2 * NRX].rearrange("j (b s) -> j b s", b=BC)
            qn3 = pk4[0:1, 2 * NRX:2 * NRX + 3]
            bo = pk4[0:1, 2 * NRX + 3:2 * NRX + 4]
            # image straight into xt (strided dst), split across two DMA
            # queues so the two halves transfer concurrently
            sy.dma_start(pksen[:], pksend.ap())
            sy.dma_start(xt[:, :, 0:NRX // 2], ximg2.ap()[:, :, 0:NRX // 2])
            nc.scalar.dma_start(xt[:, :, NRX // 2:NRX],
                                ximg2.ap()[:, :, NRX // 2:NRX])
            nc.gpsimd.dma_start(pk32[:], pk32d.ap())
            nc.gpsimd.dma_start(pk16[:], pk16d.ap())
            nc.gpsimd.dma_start(pk4[:], pk4d.ap())
            nc.gpsimd.memset(z11[:], 0.0)
            nc.gpsimd.memset(e11[:], 1e-5)
            nc.gpsimd.memset(one3[:], 1.0)
            nc.gpsimd.memset(xt[:, :, NR - 1:NR], 1.0)
            # prewarm the Sqrt activation table so the tail doesn't stall
            warm = vp.tile([1, 1], f32, tag="warm")
            nc.scalar.activation(warm[:], z11[:], AF.Sqrt, bias=z11[:])

            # ---- xt finalize: add tok emb in place (two half-row sweeps) ----
            for h0, h1 in ((0, NRX // 2), (NRX // 2, NRX)):
                nc.vector.tensor_tensor(
                    xt[:, :, h0:h1], xt[:, :, h0:h1],
                    tok[:, :, None].to_broadcast((128, KT, h1 - h0)), OP.add)

            def build_v_group(g0, g1):
                ng = g1 - g0
                sl = slice(g0, g1)
                P1r = fy[:, sl, 0, 0:BC]; P1i = fy[:, sl, 1, 0:BC]
                P2r = fy[:, sl, 2, 0:BC]; P2i = fy[:, sl, 3, 0:BC]
                shp = (128, ng, BC)
                Q1r = fy[:, sl, 0, 4:5].to_broadcast(shp)
                Q1i = fy[:, sl, 1, 4:5].to_broadcast(shp)
                Q2r = fy[:, sl, 2, 4:5].to_broadcast(shp)
                Q2i = fy[:, sl, 3, 4:5].to_broadcast(shp)
                gb = gam[:, sl, None].to_broadcast(shp)
                va = vp.tile([128, 8, BC], f32, tag="va", name="va")[:, :ng, :]
                vb = vp.tile([128, 8, BC], f32, tag="vb", name="vb")[:, :ng, :]
                vc = vp.tile([128, 8, BC], f32, tag="vc", name="vc")[:, :ng, :]
                TT = nc.vector.tensor_tensor
                TT(va[:], P1r, P2r, OP.mult)
                TT(vb[:], P1i, P2i, OP.mult)
                TT(vc[:], va[:], vb[:], OP.subtract)
                TT(vt[:, sl, 0, :, 0], vc[:], gb, OP.mult)
                TT(va[:], P1r, P2i, OP.mult)
                TT(vb[:], P1i, P2r, OP.mult)
                TT(vc[:], va[:], vb[:], OP.add)
                TT(vt[:, sl, 1, :, 0], vc[:], gb, OP.mult)
                TT(va[:], P1r, Q2r, OP.mult)
                TT(vb[:], P1i, Q2i, OP.mult)
                TT(va[:], va[:], vb[:], OP.subtract)
                TT(vb[:], P2r, Q1r, OP.mult)
                TT(vc[:], P2i, Q1i, OP.mult)
                TT(vb[:], vb[:], vc[:], OP.subtract)
                TT(va[:], va[:], vb[:], OP.add)
                TT(vt[:, sl, 0, :, 1], va[:], gb, OP.mult)
                TT(va[:], P1r, Q2i, OP.mult)
                TT(vb[:], P1i, Q2r, OP.mult)
                TT(va[:], va[:], vb[:], OP.add)
                TT(vb[:], P2r, Q1i, OP.mult)
                TT(vc[:], P2i, Q1r, OP.mult)
                TT(vb[:], vb[:], vc[:], OP.add)
                TT(va[:], va[:], vb[:], OP.add)
                TT(vt[:, sl, 1, :, 1], va[:], gb, OP.mult)
                nc.vector.tensor_copy(
                    vt[:, sl, :, :, 2],
                    v3s[:, sl, :, None].to_broadcast((128, ng, 2, BC)))

            with (
                tc.tile_pool(name="mps", bufs=4, space="PSUM") as mps,
                tc.tile_pool(name="p2ps", bufs=1, space="PSUM") as p2,
            ):
                # sensor branch -> t rows (cols NRX..NRX+BC)
                for kt in range(KT):
                    pss = mps.tile([128, 293], f32, tag="mm",
                                   name="pss")[:, 0:BC]
                    nc.tensor.matmul(pss[:],
                                     pksen[:, kt * 128:(kt + 1) * 128],
                                     pksen[:, C:C + BC], start=True, stop=True)
                    nc.vector.tensor_tensor(
                        xt[:, kt, NRX:NRX + BC], pss[:],
                        bse[:, kt:kt + 1].to_broadcast((128, BC)), OP.add)

                tps = [p2.tile([12, 2 * S], f32, tag=f"tps{h}", name=f"tps{h}")
                       for h in range(2)]

                # ---- Nyquist k=4096 (real spectra, gamma=1): opens the
                # pass-2 accumulation chains with a rank-1 update ----
                fn0 = fp.tile([1, NR], f16, tag="fn0")
                fn1 = fp.tile([1, NR], f16, tag="fn1")
                for pl, fn in ((0, fn0), (1, fn1)):
                    for (c0, nn) in CH:
                        psn = mps.tile([128, 293], f32, tag="mm",
                                       name="psn")[0:1, :]
                        for kt in range(KT):
                            nc.tensor.matmul(
                                psn[:, :nn], eny[:, kt, pl:pl + 1],
                                xt[:, kt, c0:c0 + nn],
                                start=(kt == 0), stop=(kt == KT - 1))
                        nc.scalar.copy(fn[0:1, c0:c0 + nn], psn[:, :nn])
                phin = vp.tile([1, NR], f16, tag="phin")
                nc.vector.tensor_tensor(phin[:], fn0[:], fn1[:], OP.mult)
                # vtn[b, j]: j0 = T1n*T2n, j1 = T1n*Q2n + T2n*Q1n, j2 = Q1n*Q2n
                vtn = vp.tile([1, BC, 3], f16, tag="vtn")
                ta = vp.tile([1, BC], f32, tag="ta")
                tb = vp.tile([1, BC], f32, tag="tb")
                nc.vector.tensor_tensor(ta[:], fn0[0:1, NRX:NRX + BC],
                                        fn1[0:1, NRX:NRX + BC], OP.mult)
                nc.vector.tensor_copy(vtn[:, :, 0], ta[:])
                nc.vector.tensor_tensor(
                    ta[:], fn0[0:1, NRX:NRX + BC],
                    qn3[:, 1:2].to_broadcast((1, BC)), OP.mult)
                nc.vector.tensor_tensor(
                    tb[:], fn1[0:1, NRX:NRX + BC],
                    qn3[:, 0:1].to_broadcast((1, BC)), OP.mult)
                nc.vector.tensor_tensor(ta[:], ta[:], tb[:], OP.add)
                nc.vector.tensor_copy(vtn[:, :, 1], ta[:])
                nc.vector.tensor_copy(
                    vtn[:, :, 2], qn3[:, 2:3].to_broadcast((1, BC)))
                vtn12 = vtn[:].rearrange("a b j -> a (b j)")
                for h in range(2):
                    nc.tensor.matmul(
                        tps[h][:], vtn12,
                        phin[0:1, h * 2 * S:(h + 1) * 2 * S],
                        start=True, stop=False)

                # ---- main loop over frequency tiles; pass 2 per group ----
                phis = {}
                for ft in range(NFT):
                    et = ep.tile([128, KT, 4, 128], f16, tag="et")
                    sy.dma_start(et[:], Ec.ap()[ft])
                    ftile = fp.tile([128, 4, NR], f16, tag="ftile")
                    for p in range(4):
                        for (c0, nn) in CH:
                            ps = mps.tile([128, 293], f32, tag="mm")
                            for kt in range(KT):
                                nc.tensor.matmul(
                                    ps[:, :nn], et[:, kt, p, :],
                                    xt[:, kt, c0:c0 + nn],
                                    start=(kt == 0), stop=(kt == KT - 1))
                            nc.scalar.copy(ftile[:, p, c0:c0 + nn],
                                           ps[:, :nn])
                    # persist spectra of the 5 appended rows
                    nc.scalar.copy(fy[:, ft, :, :], ftile[:, :, NRX:NR])
                    # Phi = F1 * F2 (complex)
                    t1 = vp.tile([128, NR], f16, tag="t1")
                    t2 = vp.tile([128, NR], f16, tag="t2")
                    phR = php.tile([128, NR], f16, tag="phR")
                    phI = php.tile([128, NR], f16, tag="phI")
                    TT = nc.vector.tensor_tensor
                    TT(t1[:], ftile[:, 0, :], ftile[:, 2, :], OP.mult)
                    TT(t2[:], ftile[:, 1, :], ftile[:, 3, :], OP.mult)
                    TT(phR[:], t1[:], t2[:], OP.subtract)
                    TT(t1[:], ftile[:, 0, :], ftile[:, 3, :], OP.mult)
                    TT(t2[:], ftile[:, 1, :], ftile[:, 2, :], OP.mult)
                    TT(phI[:], t1[:], t2[:], OP.add)
                    phis[ft] = (phR, phI)
                    if ft in VG:
                        g0, g1 = VG[ft]
                        build_v_group(g0, g1)
                        for fti in range(g0, g1):
                            pR, pI = phis.pop(fti)
                            fin = fti == NFT - 1
                            vR = vt[:, fti, 0].rearrange("p b j -> p (b j)")
                            vI = vt[:, fti, 1].rearrange("p b j -> p (b j)")
                            for h in range(2):
                                nc.tensor.matmul(
                                    tps[h][:], vR,
                                    pR[:, h * 2 * S:(h + 1) * 2 * S],
                                    start=False, stop=False)
                                nc.tensor.matmul(
                                    tps[h][:], vI,
                                    pI[:, h * 2 * S:(h + 1) * 2 * S],
                                    start=False, stop=fin)

                # ---- epilogue: masked wv multiply then ones-12 matmul ----
                tsb = cp.tile([12, 2, 2 * S], f32)
                uu = vp.tile([12, 2, 2 * S], f16, tag="uu")
                ip = vp.tile([1, BC, S], f32, tag="ip")
                ipf = ip[:].rearrange("a b s -> a (b s)")
                for h in range(2):
                    nc.scalar.copy(tsb[:, h, :], tps[h][:])
                    nc.vector.tensor_tensor(uu[:, h, :], tsb[:, h, :],
                                            wv12[:, h, :], OP.mult)
                    ipp = mps.tile([128, 293], f32, tag="mm",
                                   name="ipp")[0:1, 0:2 * S]
                    nc.tensor.matmul(ipp[:], one3[:], uu[:, h, :],
                                     start=True, stop=True)
                    if h == 0:
                        nc.scalar.copy(ipf[:, 0:2 * S], ipp[:])
                    else:
                        nc.vector.tensor_copy(ipf[:, 2 * S:4 * S], ipp[:])

            # ---- tail: signed sqrt, L2 normalize over s, project ----
            sgn = vp.tile([1, BC, S], f32, tag="sgn")
            nc.vector.tensor_scalar(sgn[:], ip[:], 0.0, None, OP.is_ge)
            nc.vector.tensor_scalar(sgn[:], sgn[:], 2.0, -1.0, OP.mult, OP.add)
            av = vp.tile([1, BC, S], f32, tag="av")          # |ip|
            nc.vector.tensor_tensor(av[:], ip[:], sgn[:], OP.mult)
            sq = vp.tile([1, BC, S], f32, tag="sq")          # sqrt(|ip|+1e-5)
            nc.scalar.activation(sq[:], av[:], AF.Sqrt, bias=e11[:])
            gg = vp.tile([1, BC, S], f32, tag="gg")
            nc.vector.tensor_tensor(gg[:], sgn[:], wo4[:], OP.mult)
            mm2 = vp.tile([1, BC, S], f32, tag="mm2")
            nc.vector.tensor_tensor(mm2[:], sq[:], gg[:], OP.mult)
            n2 = vp.tile([1, BC], f32, tag="n2")
            ds = vp.tile([1, BC], f32, tag="ds")
            nc.vector.tensor_reduce(n2[:], av[:],
                                    axis=mybir.AxisListType.X, op=OP.add)
            # ||bp||^2 = sum(|ip|) + S*1e-5
            nc.vector.tensor_scalar(n2[:], n2[:], S * 1e-5, None, OP.add)
            nc.vector.tensor_reduce(ds[:], mm2[:],
                                    axis=mybir.AxisListType.X, op=OP.add)
            inv2 = vp.tile([1, BC], f32, tag="inv2")
            nc.vector.reciprocal(inv2[:], n2[:])
            invn = vp.tile([1, BC], f32, tag="invn")
            nc.scalar.activation(invn[:], inv2[:], AF.Sqrt, bias=z11[:])
            res = vp.tile([1, BC], f32, tag="res")
            nc.vector.tensor_tensor(res[:], ds[:], invn[:], OP.mult)
            nc.vector.tensor_tensor(res[:], res[:],
                                    bo[:, 0:1].to_broadcast((1, BC)), OP.add)
            sy.dma_start(out_d.ap(), res[:])

    nc.compile()
    return nc


def kernel(**inputs) -> np.ndarray:
    global _PROGRAM
    if _PROGRAM is None:
        _PROGRAM = _build_program()
    nc = _PROGRAM

    consts = _host_constants(
        inputs["h1"], inputs["h2"], inputs["s1"], inputs["s2"])
    in_maps = [_host_inputs_for_core(c, inputs, consts)
               for c in range(NCORES)]

    from concourse.bass_utils import run_bass_kernel_spmd
    res = run_bass_kernel_spmd(nc, in_maps, list(range(NCORES)))
    out = np.concatenate([res.results[c]["out"][0] for c in range(NCORES)],
                         axis=0)
    return out.reshape(B, 1).astype(np.float32)


# revision 30
# speedup vs baseline: 1.0626x; 1.0055x over previous
"""Trainium2 Bass kernel for nn_CompactBilinearPoolingTSP.

The count-sketch + FFT circular-convolution pipeline collapses, via Parseval,
into dense half-spectrum DFT matmuls: F1[r,k] = sum_c X[r,c] E1[c,k] with
E1[c,k] = s1[c] exp(-2i pi k h1[c] / D) a host-precomputed constant,
Phi = F1 * F2, and ip[r] = (1/D) sum_k gamma[k] Re(Phi conj(F1y F2y)).
The sensor branch is rank-1 in s, so the y-side spectra reduce to three
per-b vectors (t rows and a ones row are appended to X so one set of matmuls
produces every needed spectrum); a second small matmul contracts Phi against
them over k.  Tail (signed sqrt, L2 normalize over s, output projection) runs
on vector/scalar engines.  Sharding: pure data parallel, batch 32 -> 4 per
core across 8 NeuronCores.

v2.1: host supplies the image pre-transposed in f16 and DMAs it straight into
the xt tile (one transfer); small constants are packed into three dram
tensors to cut DMA-issue serialization at startup; the Nyquist frequency 4096
runs as a rank-1 side path hoisted before the main loop (it opens the pass-2
PSUM accumulation chains); pass 2 is interleaved into the main loop per
8-tile group so phi tiles never persist; PSUM evacuation runs on the scalar
engine; the sqrt table is prewarmed; the tail is fused (|ip|+eps via abs_max,
single-instruction per-b reductions).
"""

import numpy as np

try:
    import concourse.bass  # noqa: F401
except ImportError:  # pragma: no cover
    import sys
    for _p in ("/opt/trn_rl_repo", "/root/.axon_site/_ro/trn_rl_repo"):
        if _p not in sys.path:
            sys.path.append(_p)

_PROGRAM = None

B, S, C, D, SN = 32, 145, 768, 8192, 64
NCORES = 8
BC = B // NCORES          # batches per core = 4
NRX = BC * S              # x rows per core = 580
NR = NRX + BC + 1         # + t rows + ones row = 585
NFT = 32                  # full freq tiles of 128 -> 4096; k=4096 separate
KP = NFT * 128
KT = C // 128             # 6 contraction tiles
CH = [(0, 293), (293, 292)]  # row chunks for matmul N
VG = {7: (0, 8), 15: (8, 16), 23: (16, 24), 29: (24, 30), 31: (30, 32)}


def _host_constants(h1, h2, s1, s2):
    """E matrices + packed small constants — derived from hashes only."""
    h1 = h1.astype(np.int64); h2 = h2.astype(np.int64)
    s1f = s1.astype(np.float64); s2f = s2.astype(np.float64)
    k = np.arange(KP)
    ang1 = (-2.0 * np.pi / D) * (h1[:, None] * k[None, :])
    ang2 = (-2.0 * np.pi / D) * (h2[:, None] * k[None, :])
    E1 = s1f[:, None] * np.exp(1j * ang1)
    E2 = s2f[:, None] * np.exp(1j * ang2)
    # planes: 0=E1r 1=E1i 2=E2r 3=E2i ; layout [NFT, 128c, KT, plane, 128f]
    E = np.stack([E1.real, E1.imag, E2.real, E2.imag], axis=0)  # [4, C, KP]
    E = E.reshape(4, KT, 128, NFT, 128)                          # [p, kt, c, ft, f]
    E = E.transpose(3, 2, 1, 0, 4)                               # [ft, c, kt, p, f]
    E = np.ascontiguousarray(E, dtype=np.float16)

    gamma = np.full(KP, 2.0)
    gamma[0] = 1.0
    gamma_sb = gamma.reshape(NFT, 128).T.astype(np.float32)      # [128, NFT]

    # v3 = gamma * (W3R, W3I), W3 = Q1*Q2, Q = ones @ E  (exact, host)
    Q1 = np.ones(C) @ E1
    Q2 = np.ones(C) @ E2
    W3 = Q1 * Q2
    v3 = np.stack([(gamma * W3.real), (gamma * W3.imag)], axis=-1)  # [KP, 2]
    v3_sb = v3.reshape(NFT, 128, 2).transpose(1, 0, 2)              # [128, NFT, 2]

    # Nyquist (k = D/2): E real, gamma = 1
    e1n = s1f * np.where(h1 % 2 == 0, 1.0, -1.0)
    e2n = s2f * np.where(h2 % 2 == 0, 1.0, -1.0)
    eny = np.stack([e1n, e2n], axis=-1).reshape(KT, 128, 2).transpose(1, 0, 2)
    qn = np.array([e1n.sum(), e2n.sum()], np.float64)               # Q1n, Q2n

    # f16 pack [128, 2*NFT + 2*KT]: v3 | eny
    pk16 = np.concatenate([v3_sb.reshape(128, 2 * NFT),
                           eny.reshape(128, 2 * KT)], axis=1)
    pk16 = np.ascontiguousarray(pk16, np.float16)
    return E, gamma_sb, pk16, qn


def _host_inputs_for_core(core, inputs, consts):
    """Per-core in_map (numpy) keyed by dram tensor names."""
    E, gamma_sb, pk16, qn = consts
    img = np.asarray(inputs["image_embeds"], np.float32)
    sensor = np.asarray(inputs["sensor"], np.float32)
    b0 = core * BC
    # [128, KT, NRX]: ximg2[p, kt, r] = img_row_r[kt*128 + p]
    ximg2 = np.ascontiguousarray(
        img[b0:b0 + BC].reshape(NRX, C).T.reshape(KT, 128, NRX)
        .transpose(1, 0, 2).astype(np.float16))

    w2 = np.asarray(inputs["W_s2"], np.float32)[:, 0]            # [S]
    beta = np.asarray(inputs["b_s2"], np.float32)                # [S]
    wv = np.stack([w2 * w2, w2 * beta, beta * beta], 0) / D      # [3, S]
    wo4 = np.broadcast_to(np.asarray(inputs["W_out"], np.float32)[0][None, :],
                          (BC, S)).reshape(NRX)
    # pk4 [12, 2*NRX + 4]: rows (3b+j) hold wv[j] in batch b's column block of
    # each 290-wide pass-2 chunk (zeros elsewhere mask the off-block garbage
    # of the batched 12-wide pass-2 matmul); row 0 cols NRX: = wo4 | qn1 qn2
    # v3n bout (all 1-partition operands at partition 0)
    pk4 = np.zeros((12, 2 * NRX + 4), np.float32)
    for b in range(BC):
        ch, off = divmod(b * S, 2 * S)          # chunk index, offset in chunk
        for j in range(3):
            pk4[3 * b + j, ch * 2 * S + off:ch * 2 * S + off + S] = wv[j]
    pk4[0, NRX:2 * NRX] = wo4
    pk4[0, 2 * NRX + 0] = qn[0]
    pk4[0, 2 * NRX + 1] = qn[1]
    pk4[0, 2 * NRX + 2] = np.float32(qn[0] * qn[1])
    pk4[0, 2 * NRX + 3] = np.asarray(inputs["b_out"], np.float32).ravel()[0]

    tokv = np.asarray(inputs["tok_emb"], np.float32)[1].reshape(KT, 128).T
    bsen = np.asarray(inputs["b_sensor"], np.float32).reshape(KT, 128).T
    pk32 = np.concatenate([gamma_sb, tokv, bsen], axis=1)        # [128, NFT+12]

    # sensor pack [SN, C + BC] f16: wsensT | sensT
    wsensT = np.asarray(inputs["W_sensor"], np.float32).T        # [SN, C]
    sensT = sensor[b0:b0 + BC, 0, :].T                           # [SN, BC]
    pksen = np.ascontiguousarray(
        np.concatenate([wsensT, sensT], axis=1).astype(np.float16))

    return {
        "ximg2": ximg2,
        "Econst": E,
        "pk32": np.ascontiguousarray(pk32, np.float32),
        "pk16": pk16,
        "pk4": np.ascontiguousarray(pk4),
        "pksen": pksen,
    }


def _build_program():
    import concourse.tile as tile
    from concourse import bacc, mybir

    f16 = mybir.dt.float16
    f32 = mybir.dt.float32
    OP = mybir.AluOpType
    AF = mybir.ActivationFunctionType

    nc = bacc.Bacc("TRN2", target_bir_lowering=False, debug=False,
                   num_devices=NCORES)

    ximg2 = nc.dram_tensor("ximg2", [128, KT, NRX], f16, kind="ExternalInput")
    Ec = nc.dram_tensor("Econst", [NFT, 128, KT, 4, 128], f16,
                        kind="ExternalInput")
    pk32d = nc.dram_tensor("pk32", [128, NFT + 2 * KT], f32,
                           kind="ExternalInput")
    pk16d = nc.dram_tensor("pk16", [128, 2 * NFT + 2 * KT], f16,
                           kind="ExternalInput")
    pk4d = nc.dram_tensor("pk4", [12, 2 * NRX + 4], f32, kind="ExternalInput")
    pksend = nc.dram_tensor("pksen", [SN, C + BC], f16, kind="ExternalInput")
    out_d = nc.dram_tensor("out", [1, BC], f32, kind="ExternalOutput")

    with tile.TileContext(nc) as tc:
        with (
            tc.tile_pool(name="const", bufs=1) as cp,
            tc.tile_pool(name="estream", bufs=3) as ep,
            tc.tile_pool(name="fplane", bufs=2) as fp,
            tc.tile_pool(name="phip", bufs=10) as php,
            tc.tile_pool(name="vtmp", bufs=2) as vp,
        ):
            # ---- persistent tiles ----
            xt = cp.tile([128, KT, NR], f16)          # rows^T (c on partitions)
            fy = cp.tile([128, NFT, 4, 5], f16)       # spectra of t rows + ones
            vt = cp.tile([128, NFT, 2, BC, 3], f16)   # lhsT for pass 2
            pk32 = cp.tile([128, NFT + 2 * KT], f32)
            pk16 = cp.tile([128, 2 * NFT + 2 * KT], f16)
            pk4 = cp.tile([12, 2 * NRX + 4], f32)
            pksen = cp.tile([SN, C + BC], f16)
            z11 = cp.tile([1, 1], f32)
            e11 = cp.tile([1, 1], f32)
            one3 = cp.tile([12, 1], f16)
            sy = nc.sync
            gam = pk32[:, 0:NFT]
            tok = pk32[:, NFT:NFT + KT]
            bse = pk32[:, NFT + KT:NFT + 2 * KT]
            v3s = pk16[:, 0:2 * NFT].rearrange("p (ft two) -> p ft two", two=2)
            eny = pk16[:, 2 * NFT:].rearrange("p (kt two) -> p kt two", two=2)
            wv12 = pk4[0:12, 0:NRX].rearrange("p (h c) -> p h c", h=2)
            wo4 = pk4[0:1, NRX:2 * NRX].rearrange("j (b s) -> j b s", b=BC)
            qn3 = pk4[0:1, 2 * NRX:2 * NRX + 3]
            bo = pk4[0:1, 2 * NRX + 3:2 * NRX + 4]
            # image straight into xt (strided dst), split across two DMA
            # queues so the two halves transfer concurrently
            sy.dma_start(pksen[:], pksend.ap())
            sy.dma_start(xt[:, :, 0:NRX // 2], ximg2.ap()[:, :, 0:NRX // 2])
            nc.scalar.dma_start(xt[:, :, NRX // 2:NRX],
                                ximg2.ap()[:, :, NRX // 2:NRX])
            nc.gpsimd.dma_start(pk32[:], pk32d.ap())
            nc.gpsimd.dma_start(pk16[:], pk16d.ap())
            nc.gpsimd.dma_start(pk4[:], pk4d.ap())
            nc.gpsimd.memset(z11[:], 0.0)
            nc.gpsimd.memset(e11[:], 1e-5)
            nc.gpsimd.memset(one3[:], 1.0)
            nc.gpsimd.memset(xt[:, :, NR - 1:NR], 1.0)
            # prewarm the Sqrt activation table so the tail doesn't stall
            warm = vp.tile([1, 1], f32, tag="warm")
            nc.scalar.activation(warm[:], z11[:], AF.Sqrt, bias=z11[:])

            # ---- xt finalize: add tok emb in place (two half-row sweeps) ----
            for h0, h1 in ((0, NRX // 2), (NRX // 2, NRX)):
                nc.vector.tensor_tensor(
                    xt[:, :, h0:h1], xt[:, :, h0:h1],
                    tok[:, :, None].to_broadcast((128, KT, h1 - h0)), OP.add)

            def build_v_group(g0, g1):
                ng = g1 - g0
                sl = slice(g0, g1)
                P1r = fy[:, sl, 0, 0:BC]; P1i = fy[:, sl, 1, 0:BC]
                P2r = fy[:, sl, 2, 0:BC]; P2i = fy[:, sl, 3, 0:BC]
                shp = (128, ng, BC)
                Q1r = fy[:, sl, 0, 4:5].to_broadcast(shp)
                Q1i = fy[:, sl, 1, 4:5].to_broadcast(shp)
                Q2r = fy[:, sl, 2, 4:5].to_broadcast(shp)
                Q2i = fy[:, sl, 3, 4:5].to_broadcast(shp)
                gb = gam[:, sl, None].to_broadcast(shp)
                va = vp.tile([128, 8, BC], f32, tag="va", name="va")[:, :ng, :]
                vb = vp.tile([128, 8, BC], f32, tag="vb", name="vb")[:, :ng, :]
                vc = vp.tile([128, 8, BC], f32, tag="vc", name="vc")[:, :ng, :]
                TT = nc.vector.tensor_tensor
                TT(va[:], P1r, P2r, OP.mult)
                TT(vb[:], P1i, P2i, OP.mult)
                TT(vc[:], va[:], vb[:], OP.subtract)
                TT(vt[:, sl, 0, :, 0], vc[:], gb, OP.mult)
                TT(va[:], P1r, P2i, OP.mult)
                TT(vb[:], P1i, P2r, OP.mult)
                TT(vc[:], va[:], vb[:], OP.add)
                TT(vt[:, sl, 1, :, 0], vc[:], gb, OP.mult)
                TT(va[:], P1r, Q2r, OP.mult)
                TT(vb[:], P1i, Q2i, OP.mult)
                TT(va[:], va[:], vb[:], OP.subtract)
                TT(vb[:], P2r, Q1r, OP.mult)
                TT(vc[:], P2i, Q1i, OP.mult)
                TT(vb[:], vb[:], vc[:], OP.subtract)
                TT(va[:], va[:], vb[:], OP.add)
                TT(vt[:, sl, 0, :, 1], va[:], gb, OP.mult)
                TT(va[:], P1r, Q2i, OP.mult)
                TT(vb[:], P1i, Q2r, OP.mult)
                TT(va[:], va[:], vb[:], OP.add)
                TT(vb[:], P2r, Q1i, OP.mult)
                TT(vc[:], P2i, Q1r, OP.mult)
                TT(vb[:], vb[:], vc[:], OP.add)
                TT(va[:], va[:], vb[:], OP.add)
                TT(vt[:, sl, 1, :, 1], va[:], gb, OP.mult)
                nc.vector.tensor_copy(
                    vt[:, sl, :, :, 2],
                    v3s[:, sl, :, None].to_broadcast((128, ng, 2, BC)))

            with (
                tc.tile_pool(name="mps", bufs=6, space="PSUM") as mps,
                tc.tile_pool(name="p2ps", bufs=1, space="PSUM") as p2,
            ):
                # sensor branch -> t rows (cols NRX..NRX+BC)
                for kt in range(KT):
                    pss = mps.tile([128, 293], f32, tag="mm",
                                   name="pss")[:, 0:BC]
                    nc.tensor.matmul(pss[:],
                                     pksen[:, kt * 128:(kt + 1) * 128],
                                     pksen[:, C:C + BC], start=True, stop=True)
                    nc.vector.tensor_tensor(
                        xt[:, kt, NRX:NRX + BC], pss[:],
                        bse[:, kt:kt + 1].to_broadcast((128, BC)), OP.add)

                tps = [p2.tile([12, 2 * S], f32, tag=f"tps{h}", name=f"tps{h}")
                       for h in range(2)]

                # ---- Nyquist k=4096 (real spectra, gamma=1): opens the
                # pass-2 accumulation chains with a rank-1 update ----
                fn0 = fp.tile([1, NR], f16, tag="fn0")
                fn1 = fp.tile([1, NR], f16, tag="fn1")
                for pl, fn in ((0, fn0), (1, fn1)):
                    for (c0, nn) in CH:
                        psn = mps.tile([128, 293], f32, tag="mm",
                                       name="psn")[0:1, :]
                        for kt in range(KT):
                            nc.tensor.matmul(
                                psn[:, :nn], eny[:, kt, pl:pl + 1],
                                xt[:, kt, c0:c0 + nn],
                                start=(kt == 0), stop=(kt == KT - 1))
                        nc.scalar.copy(fn[0:1, c0:c0 + nn], psn[:, :nn])
                phin = vp.tile([1, NR], f16, tag="phin")
                nc.vector.tensor_tensor(phin[:], fn0[:], fn1[:], OP.mult)
                # vtn[b, j]: j0 = T1n*T2n, j1 = T1n*Q2n + T2n*Q1n, j2 = Q1n*Q2n
                vtn = vp.tile([1, BC, 3], f16, tag="vtn")
                ta = vp.tile([1, BC], f32, tag="ta")
                tb = vp.tile([1, BC], f32, tag="tb")
                nc.vector.tensor_tensor(ta[:], fn0[0:1, NRX:NRX + BC],
                                        fn1[0:1, NRX:NRX + BC], OP.mult)
                nc.vector.tensor_copy(vtn[:, :, 0], ta[:])
                nc.vector.tensor_tensor(
                    ta[:], fn0[0:1, NRX:NRX + BC],
                    qn3[:, 1:2].to_broadcast((1, BC)), OP.mult)
                nc.vector.tensor_tensor(
                    tb[:], fn1[0:1, NRX:NRX + BC],
                    qn3[:, 0:1].to_broadcast((1, BC)), OP.mult)
                nc.vector.tensor_tensor(ta[:], ta[:], tb[:], OP.add)
                nc.vector.tensor_copy(vtn[:, :, 1], ta[:])
                nc.vector.tensor_copy(
                    vtn[:, :, 2], qn3[:, 2:3].to_broadcast((1, BC)))
                vtn12 = vtn[:].rearrange("a b j -> a (b j)")
                for h in range(2):
                    nc.tensor.matmul(
                        tps[h][:], vtn12,
                        phin[0:1, h * 2 * S:(h + 1) * 2 * S],
                        start=True, stop=False)

                # ---- main loop over frequency tiles; pass 2 per group ----
                phis = {}
                for ft in range(NFT):
                    et = ep.tile([128, KT, 4, 128], f16, tag="et")
                    sy.dma_start(et[:], Ec.ap()[ft])
                    phR = php.tile([128, NR], f16, tag="phR")
                    phI = php.tile([128, NR], f16, tag="phI")
                    TT = nc.vector.tensor_tensor
                    for (c0, nn) in CH:
                        pp4 = []
                        for p in range(4):
                            ps = mps.tile([128, 293], f32, tag="mm")
                            for kt in range(KT):
                                nc.tensor.matmul(
                                    ps[:, :nn], et[:, kt, p, :],
                                    xt[:, kt, c0:c0 + nn],
                                    start=(kt == 0), stop=(kt == KT - 1))
                            pp4.append(ps)
                        if c0 + nn == NR:   # tail chunk: persist t/ones rows
                            for p in range(4):
                                nc.scalar.copy(fy[:, ft, p, :],
                                               pp4[p][:, NR - 5 - c0:nn])
                        # evacuate F1 planes; products take one PSUM operand
                        sb0 = vp.tile([128, 293], f16, tag="sb0")
                        sb1 = vp.tile([128, 293], f16, tag="sb1")
                        nc.scalar.copy(sb0[:, :nn], pp4[0][:, :nn])
                        nc.scalar.copy(sb1[:, :nn], pp4[1][:, :nn])
                        t1 = vp.tile([128, 293], f16, tag="t1")
                        t2 = vp.tile([128, 293], f16, tag="t2")
                        sl = slice(c0, c0 + nn)
                        TT(t1[:, :nn], sb0[:, :nn], pp4[2][:, :nn], OP.mult)
                        TT(t2[:, :nn], sb1[:, :nn], pp4[3][:, :nn], OP.mult)
                        TT(phR[:, sl], t1[:, :nn], t2[:, :nn], OP.subtract)
                        TT(t1[:, :nn], sb0[:, :nn], pp4[3][:, :nn], OP.mult)
                        TT(t2[:, :nn], sb1[:, :nn], pp4[2][:, :nn], OP.mult)
                        TT(phI[:, sl], t1[:, :nn], t2[:, :nn], OP.add)
                    phis[ft] = (phR, phI)
                    if ft in VG:
                        g0, g1 = VG[ft]
                        build_v_group(g0, g1)
                        for fti in range(g0, g1):
                            pR, pI = phis.pop(fti)
                            fin = fti == NFT - 1
                            vR = vt[:, fti, 0].rearrange("p b j -> p (b j)")
                            vI = vt[:, fti, 1].rearrange("p b j -> p (b j)")
                            for h in range(2):
                                nc.tensor.matmul(
                                    tps[h][:], vR,
                                    pR[:, h * 2 * S:(h + 1) * 2 * S],
                                    start=False, stop=False)
                                nc.tensor.matmul(
                                    tps[h][:], vI,
                                    pI[:, h * 2 * S:(h + 1) * 2 * S],
                                    start=False, stop=fin)

                # ---- epilogue: masked wv multiply then ones-12 matmul ----
                tsb = cp.tile([12, 2, 2 * S], f32)
                uu = vp.tile([12, 2, 2 * S], f16, tag="uu")
                ip = vp.tile([1, BC, S], f32, tag="ip")
                ipf = ip[:].rearrange("a b s -> a (b s)")
                for h in range(2):
                    nc.scalar.copy(tsb[:, h, :], tps[h][:])
                    nc.vector.tensor_tensor(uu[:, h, :], tsb[:, h, :],
                                            wv12[:, h, :], OP.mult)
                    ipp = mps.tile([128, 293], f32, tag="mm",
                                   name="ipp")[0:1, 0:2 * S]
                    nc.tensor.matmul(ipp[:], one3[:], uu[:, h, :],
                                     start=True, stop=True)
                    if h == 0:
                        nc.scalar.copy(ipf[:, 0:2 * S], ipp[:])
                    else:
                        nc.vector.tensor_copy(ipf[:, 2 * S:4 * S], ipp[:])

            # ---- tail: signed sqrt, L2 normalize over s, project ----
            sgn = vp.tile([1, BC, S], f32, tag="sgn")
            nc.vector.tensor_scalar(sgn[:], ip[:], 0.0, None, OP.is_ge)
            nc.vector.tensor_scalar(sgn[:], sgn[:], 2.0, -1.0, OP.mult, OP.add)
            av = vp.tile([1, BC, S], f32, tag="av")          # |ip|
            nc.vector.tensor_tensor(av[:], ip[:], sgn[:], OP.mult)
            sq = vp.tile([1, BC, S], f32, tag="sq")          # sqrt(|ip|+1e-5)
            nc.scalar.activation(sq[:], av[:], AF.Sqrt, bias=e11[:])
            gg = vp.tile([1, BC, S], f32, tag="gg")
            nc.vector.tensor_tensor(gg[:], sgn[:], wo4[:], OP.mult)
            mm2 = vp.tile([1, BC, S], f32, tag="mm2")
            nc.vector.tensor_tensor(mm2[:], sq[:], gg[:], OP.mult)
            n2 = vp.tile([1, BC], f32, tag="n2")
            ds = vp.tile([1, BC], f32, tag="ds")
            nc.vector.tensor_reduce(n2[:], av[:],
                                    axis=mybir.AxisListType.X, op=OP.add)
            # ||bp||^2 = sum(|ip|) + S*1e-5
            nc.vector.tensor_scalar(n2[:], n2[:], S * 1e-5, None, OP.add)
            nc.vector.tensor_reduce(ds[:], mm2[:],
                                    axis=mybir.AxisListType.X, op=OP.add)
            inv2 = vp.tile([1, BC], f32, tag="inv2")
            nc.vector.reciprocal(inv2[:], n2[:])
            invn = vp.tile([1, BC], f32, tag="invn")
            nc.scalar.activation(invn[:], inv2[:], AF.Sqrt, bias=z11[:])
            res = vp.tile([1, BC], f32, tag="res")
            nc.vector.tensor_tensor(res[:], ds[:], invn[:], OP.mult)
            nc.vector.tensor_tensor(res[:], res[:],
                                    bo[:, 0:1].to_broadcast((1, BC)), OP.add)
            sy.dma_start(out_d.ap(), res[:])

    nc.compile()
    return nc


def kernel(**inputs) -> np.ndarray:
    global _PROGRAM
    if _PROGRAM is None:
        _PROGRAM = _build_program()
    nc = _PROGRAM

    consts = _host_constants(
        inputs["h1"], inputs["h2"], inputs["s1"], inputs["s2"])
    in_maps = [_host_inputs_for_core(c, inputs, consts)
               for c in range(NCORES)]

    from concourse.bass_utils import run_bass_kernel_spmd
    res = run_bass_kernel_spmd(nc, in_maps, list(range(NCORES)))
    out = np.concatenate([res.results[c]["out"][0] for c in range(NCORES)],
                         axis=0)
    return out.reshape(B, 1).astype(np.float32)
